# revision 35
# baseline (speedup 1.0000x reference)
"""Multi-head causal attention with RoPE on 8 TRN2 NeuronCores.

Problem: B=2, T=2048, D=1024, H=16 heads, head_dim=64.
  out = softmax(mask(rope(x@Wq.T) @ rope(x@Wk.T).T / 8)) @ (x@Wv.T) @ Wo.T

Sharding: tensor-parallel over heads. Core c owns heads {2c, 2c+1}:
  - computes Q/K/V projections for its 2 heads over all 4096 tokens
    (x pre-transposed/pre-tiled on host; the 1/sqrt(hd) scale is folded
    into Wq),
  - RoPE via a block-diagonal rotation matmul + cos/sin elementwise,
  - causal flash-style attention in transposed layout (scores^T [k, q]
    tiles, exp on ScalarE, lower-triangle tiles only, row-sums via an
    appended ones-column on V),
  - an AllToAll (2 x 1MB bf16, split by local head) redistributes
    attention outputs from head-sharded to row-sharded,
  - row-parallel output projection, contracted in two K=64 halves so
    the first half overlaps the second AllToAll: core c computes output
    rows [512c, 512(c+1)) of the flattened [4096, 1024] output.

Matmuls run as float32r (fp32 storage, TF32-like multiply, 1 cyc/row at
N>=256); PSUM accumulates fp32. Two hardware quirks shape the code: the
PE only pipelines back-to-back matmuls whose contraction dim K matches
(K-reconfig flushes it), so scores matmuls use a per-head zero-padded
rope(K) to stay K=128 and V-transposes are batched; and the projection
phase (PE-heavy, idle ScalarE) is interleaved with the attention phase
(ScalarE-exp-bound) so both engines stay busy.
"""
import sys

sys.path.insert(0, "/opt/trn_rl_repo")

import numpy as np

from concourse import bacc, mybir, tile
from concourse import bass_utils

N_CORES = 8
B, T, D, H = 2, 2048, 1024, 16
HD = D // H              # 64
HPC = H // N_CORES       # 2 heads per core
BT = B * T               # 4096
NF = D // 128            # 8 feature chunks
NTC = BT // 512          # 8 t-chunks of 512
QCHUNK = 512
ROWS_PER_CORE = BT // N_CORES  # 512 output rows per core

F32 = mybir.dt.float32
F32R = mybir.dt.float32r
BF16 = mybir.dt.bfloat16

_CACHE = {}


def _rot_matrix():
    """R2 = blockdiag(R, R), R@u = rotate_half(u) per 64-dim head."""
    half = HD // 2
    R = np.zeros((HD, HD), dtype=np.float32)
    for i in range(half):
        R[i, i + half] = -1.0
        R[i + half, i] = 1.0
    R2 = np.zeros((2 * HD, 2 * HD), dtype=np.float32)
    R2[:HD, :HD] = R
    R2[HD:, HD:] = R
    return R2


def build(debug=False):
    nc = bacc.Bacc("TRN2", target_bir_lowering=False, debug=False,
                   num_devices=N_CORES)

    # ---- DRAM parameters (per-core shards, host-prepped layouts) ----
    xt = nc.declare_dram_parameter("xt", [NTC, 128, NF, 512], F32, isOutput=False)
    wq_t = nc.declare_dram_parameter("wq_t", [128, NF, 128], F32, isOutput=False)
    wk_t = nc.declare_dram_parameter("wk_t", [128, NF, 128], F32, isOutput=False)
    wv_t = nc.declare_dram_parameter("wv_t", [128, NF, 128], F32, isOutput=False)
    wo_t = nc.declare_dram_parameter("wo_t", [128, NF, D], BF16, isOutput=False)
    cos2 = nc.declare_dram_parameter("cos2", [128, T], F32, isOutput=False)
    sin2 = nc.declare_dram_parameter("sin2", [128, T], F32, isOutput=False)
    rot2t = nc.declare_dram_parameter("rot2t", [128, 128], F32, isOutput=False)
    ident = nc.declare_dram_parameter("ident", [128, 128], F32, isOutput=False)
    trimask = nc.declare_dram_parameter("trimask", [128, 128], F32, isOutput=False)
    out = nc.declare_dram_parameter("out", [ROWS_PER_CORE, D], F32, isOutput=True)
    if debug:
        dbg_qrope = nc.declare_dram_parameter("dbg_qrope", [128, BT], F32, isOutput=True)
        dbg_krope = nc.declare_dram_parameter("dbg_krope", [128, HPC * BT], F32, isOutput=True)
        dbg_vall = nc.declare_dram_parameter("dbg_vall", [128, B * HPC * 16 * 65], F32, isOutput=True)
        dbg_attout = nc.declare_dram_parameter("dbg_attout", [64, HPC * BT], F32, isOutput=True)
        dbg_attall = nc.declare_dram_parameter("dbg_attall", [128, N_CORES * QCHUNK], F32, isOutput=True)

    with tile.TileContext(nc) as tc, nc.allow_low_precision(reason="f32r compute"):
        with (
            tc.tile_pool(name="consts", bufs=1) as cpool,
            tc.tile_pool(name="work", bufs=1) as wpool,
            tc.tile_pool(name="psum", bufs=1, space="PSUM") as ppool,
            tc.tile_pool(name="dram", bufs=1, space="DRAM") as dpool,
        ):
            # ---- persistent tensors ----
            rot_sb = cpool.tile([128, 128], F32R, tag="rot")
            id_sb = cpool.tile([128, 128], F32R, tag="ident")
            tri_sb = cpool.tile([128, 128], F32R, tag="tri")
            cos_sb = cpool.tile([128, T], F32R, tag="cos")
            sin_sb = cpool.tile([128, T], F32R, tag="sin")
            wo_sb = cpool.tile([128, NF, D], BF16, tag="wo")
            qrope = cpool.tile([128, BT], F32R, tag="qrope")
            # K rope, zero-padded per head so scores matmuls keep K=128
            krope = cpool.tile([128, HPC, BT], F32R, tag="krope")
            # V per (b, h): [128 t-part, 16 t-tiles, 65] (col 64 = ones)
            v_all = cpool.tile([128, B, HPC, T // 128, 65], F32R, tag="v_all")
            att_out = cpool.tile([64, HPC, BT], BF16, tag="att_out")
            att_alls = [cpool.tile([128, N_CORES, QCHUNK], BF16,
                                   tag=f"att_all{h}", name=f"att_all{h}")
                        for h in range(HPC)]
            ostage = cpool.tile([128, 4, D], BF16, tag="ostage")

            wq_sb = cpool.tile([128, NF, 128], F32R, tag="wq")
            wk_sb = cpool.tile([128, NF, 128], F32R, tag="wk")
            wv_sb = cpool.tile([128, NF, 128], F32R, tag="wv")

            a2a_in = [dpool.tile([N_CORES, 64, 512], BF16, tag=f"a2a_in{h}",
                                 name=f"a2a_in{h}")
                      for h in range(HPC)]
            a2a_out = [dpool.tile([N_CORES, 64, 512], BF16, tag=f"a2a_out{h}",
                                  name=f"a2a_out{h}")
                       for h in range(HPC)]

            # ---- DMA loads; order matters: the first projection matmul
            # needs only wq f-chunk 0 + xt chunk-0 f-chunk 0 (~320KB), so
            # those are split into their own transfers to start the PE early
            nc.sync.dma_start(wq_sb[:, 0:1, :], wq_t[:, 0:1, :].bitcast(F32R))

            def load_xt_half(j, half, nsplit=1):
                """One 512-token, 4-feature-chunk half of x^T (contiguous)."""
                xh = wpool.tile([128, NF // 2, 512], F32R, tag="xt", bufs=3,
                                name="xh")
                c0 = half * 4
                step = 4 // nsplit
                for cc in range(0, 4, step):
                    nc.sync.dma_start(
                        xh[:, cc:cc + step, :],
                        xt[j, :, c0 + cc:c0 + cc + step, :].bitcast(F32R))
                return xh

            xt_pre = [load_xt_half(0, 0, nsplit=4), load_xt_half(0, 1)]
            nc.sync.dma_start(wq_sb[:, 1:8, :], wq_t[:, 1:8, :].bitcast(F32R))
            # wk/wv on the otherwise-empty gpsimd queue (ahead of wo) so the
            # Sync queue streams xt chunks back-to-back
            nc.gpsimd.dma_start(wk_sb[:], wk_t[:].bitcast(F32R))
            nc.gpsimd.dma_start(wv_sb[:], wv_t[:].bitcast(F32R))
            nc.scalar.dma_start(rot_sb[:], rot2t[:].bitcast(F32R))
            nc.scalar.dma_start(cos_sb[:], cos2[:].bitcast(F32R))
            nc.scalar.dma_start(sin_sb[:], sin2[:].bitcast(F32R))
            nc.scalar.dma_start(id_sb[:], ident[:].bitcast(F32R))
            nc.scalar.dma_start(tri_sb[:], trimask[:].bitcast(F32R))
            nc.gpsimd.dma_start(wo_sb[:], wo_t[:])

            # zero the pad halves of krope; ones col of v_all
            nc.vector.memset(krope[64:128, 0, :].bitcast(F32), 0.0)
            nc.vector.memset(krope[0:64, 1, :].bitcast(F32), 0.0)
            nc.vector.memset(v_all[:, :, :, :, 64].bitcast(F32), 1.0)

            # ---- phase A pieces ----
            def emit_a(j, xh01=None):
                """Projection chunk j; returns state for rope/vt laggards."""
                xh = xh01 or [load_xt_half(j, 0), load_xt_half(j, 1)]
                ps_q = ppool.tile([128, 512], F32, tag="pP", bufs=3)
                ps_k = ppool.tile([128, 512], F32, tag="pP", bufs=3,
                                  name="ps_k")
                ps_v = ppool.tile([128, 512], F32, tag="pP", bufs=3,
                                  name="ps_v")
                for f in range(NF):
                    st, sp = (f == 0), (f == NF - 1)
                    src = xh[f // 4][:, f % 4, :]
                    nc.tensor.matmul(ps_q[:], wq_sb[:, f, :], src,
                                     start=st, stop=sp)
                    nc.tensor.matmul(ps_k[:], wk_sb[:, f, :], src,
                                     start=st, stop=sp)
                    nc.tensor.matmul(ps_v[:], wv_sb[:, f, :], src,
                                     start=st, stop=sp)
                qT = wpool.tile([128, 512], F32R, tag="qT", bufs=1)
                kT = wpool.tile([128, 512], F32R, tag="kT", bufs=1)
                vT = wpool.tile([128, 512], F32R, tag="vT", bufs=2)
                nc.vector.tensor_copy(qT[:], ps_q[:])
                nc.vector.tensor_copy(kT[:], ps_k[:])
                nc.vector.tensor_copy(vT[:], ps_v[:])
                # rotation matmuls (same K=128/N=512 shape as projections)
                ps_rq = ppool.tile([128, 512], F32, tag="pB", bufs=2,
                                   name="ps_rq")
                nc.tensor.matmul(ps_rq[:], rot_sb[:], qT[:],
                                 start=True, stop=True)
                ps_rk = ppool.tile([128, 512], F32, tag="pB", bufs=2,
                                   name="ps_rk")
                nc.tensor.matmul(ps_rk[:], rot_sb[:], kT[:],
                                 start=True, stop=True)
                # rope combines on DVE
                tl = (j % 4) * 512
                J = slice(j * 512, (j + 1) * 512)
                TL = slice(tl, tl + 512)
                tmp = wpool.tile([128, 512], F32R, tag="ropetmp", bufs=2,
                                 name="tmp")
                nc.vector.tensor_mul(tmp[:], ps_rq[:], sin_sb[:, TL])
                nc.vector.tensor_mul(qrope[:, J], qT[:], cos_sb[:, TL])
                nc.vector.tensor_add(qrope[:, J], qrope[:, J], tmp[:])
                tmpk = wpool.tile([128, 512], F32R, tag="ropetmp", bufs=2,
                                  name="tmpk")
                nc.vector.tensor_mul(tmpk[:], ps_rk[:], sin_sb[:, TL])
                for h in range(HPC):
                    hs = slice(h * 64, (h + 1) * 64)
                    nc.vector.tensor_mul(krope[hs, h, J], kT[hs, :],
                                         cos_sb[hs, TL])
                    nc.vector.tensor_add(krope[hs, h, J], krope[hs, h, J],
                                         tmpk[hs, :])
                return (j, vT)

            def v_transposes(j, vT):
                b = j // 4
                for h in range(HPC):
                    hs = slice(h * 64, (h + 1) * 64)
                    for tt in range(4):
                        ps_t = ppool.tile([128, 64], F32R, tag="pB",
                                          bufs=2, name="ps_t")
                        nc.tensor.transpose(
                            ps_t[:, :],
                            vT[hs, tt * 128:(tt + 1) * 128],
                            id_sb[hs, hs],
                        )
                        nc.scalar.copy(
                            v_all[:, b, h, (j % 4) * 4 + tt, 0:64], ps_t[:])

            # ---- phase B pieces ----
            def scores_mm(h, base, q0, kt):
                k0 = kt * 128
                ps_s = ppool.tile([128, 512], F32, tag="pS", bufs=3,
                                  name="ps_s")
                nc.tensor.matmul(
                    ps_s[:],
                    krope[:, h, base + k0:base + k0 + 128],
                    qrope[:, base + q0:base + q0 + 512],
                    start=True, stop=True,
                )
                return ps_s

            def exp_mask(ps_s, n_full, kt):
                ae = wpool.tile([128, 512], F32R, tag="attexp", bufs=3,
                                name="ae")
                if kt < n_full:
                    nc.scalar.activation(
                        ae[:], ps_s[:], mybir.ActivationFunctionType.Exp)
                else:
                    v = kt - n_full
                    nc.scalar.activation(
                        ae[:, v * 128:512], ps_s[:, v * 128:512],
                        mybir.ActivationFunctionType.Exp)
                    nc.vector.tensor_mul(
                        ae[:, v * 128:(v + 1) * 128],
                        ae[:, v * 128:(v + 1) * 128],
                        tri_sb[:],
                    )
                    if v > 0:
                        nc.vector.memset(ae[:, 0:v * 128].bitcast(F32), 0.0)
                return ae

            def emit_b(h, b, qc):
                base = b * T
                q0 = qc * QCHUNK
                n_full = q0 // 128
                n_kt = n_full + 4
                attv = ppool.tile([65, 512], F32, tag="pB", bufs=2)
                PIPE = 3
                pend_s = [scores_mm(h, base, q0, kt)
                          for kt in range(min(PIPE, n_kt))]
                for kt in range(n_kt):
                    ae = exp_mask(pend_s[kt], n_full, kt)
                    if kt + PIPE < n_kt:
                        pend_s.append(scores_mm(h, base, q0, kt + PIPE))
                    nc.tensor.matmul(
                        attv[:], v_all[:, b, h, kt, :], ae[:],
                        start=(kt == 0), stop=(kt == n_kt - 1),
                    )
                # stage the unnormalized output + reciprocal of row-sums,
                # releasing the PSUM slot via DVE only; the DMA/broadcast/
                # multiply normalization chain is deferred (flushed in
                # batches) so its latency never blocks the DVE queue that
                # feeds attention.
                J = slice(base + q0, base + q0 + 512)
                nc.vector.tensor_copy(att_out[:, h, J], attv[0:64, :])
                rcp = wpool.tile([65, 512], F32, tag="rcp", bufs=2)
                # copy the sum row out first (fast) so the PSUM slot frees
                # without waiting for the slow reciprocal
                nc.vector.tensor_copy(rcp[64:65, :], attv[64:65, :])
                nc.vector.reciprocal(rcp[64:65, :], rcp[64:65, :])

                def norm(h=h, J=J, rcp=rcp):
                    rcp0 = wpool.tile([1, 512], F32, tag="rcp0", bufs=2)
                    nc.sync.dma_start(rcp0[:], rcp[64:65, :])
                    brcp = wpool.tile([64, 512], F32, tag="brcp", bufs=3)
                    nc.gpsimd.partition_broadcast(brcp[:], rcp0[:])
                    nc.vector.tensor_mul(
                        att_out[:, h, J], att_out[:, h, J], brcp[:])
                deferred_norms.append(norm)

            def flush_norms():
                for fn in deferred_norms:
                    fn()
                deferred_norms.clear()

            def emit_a2a(h, half_only=False):
                hs = slice(h * 64, (h + 1) * 64)
                if half_only:
                    nc.sync.dma_start(
                        a2a_in[h][4:8].transpose([1, 0, 2]),
                        att_out[:, h, T:].rearrange("p (s q) -> p s q", s=4),
                    )
                else:
                    nc.sync.dma_start(
                        a2a_in[h][:].transpose([1, 0, 2]),
                        att_out[:, h, :].rearrange("p (s q) -> p s q",
                                                   s=N_CORES),
                    )
                nc.gpsimd.collective_compute(
                    "AllToAll", mybir.AluOpType.bypass,
                    replica_groups=[list(range(N_CORES))],
                    ins=[a2a_in[h].opt()],
                    outs=[a2a_out[h].opt()],
                )
                # on gpsimd so this long-waiting DMA never heads the Sync
                # queue in front of small latency-critical transfers; each
                # head has its own tile so the h0 projection can't pick up
                # a false dependency on the h1 A2A write
                nc.gpsimd.dma_start(
                    att_alls[h][hs, :, :],
                    a2a_out[h][:].transpose([1, 0, 2]),
                )

            def proj_half(hl, oc, s):
                lo = slice(hl * 64, (hl + 1) * 64)
                ps_o = ppool.tile([128, 512], F32, tag="pP", bufs=3,
                                  name="ps_o")
                for c in range(N_CORES):
                    nc.tensor.matmul(
                        ps_o[:],
                        att_alls[hl][lo, c, s * 128:(s + 1) * 128],
                        wo_sb[lo, c, oc * 512:(oc + 1) * 512],
                        start=(c == 0), stop=(c == N_CORES - 1),
                    )
                return ps_o

            # ---- interleaved schedule ----
            # A chunks 0-3 produce batch 0; B(h0,b0) then interleaves with
            # A chunks 4-7 (batch 1), keeping ScalarE and PE both busy.
            deferred_norms = []
            pend = emit_a(0, xt_pre)
            for j in range(1, 4):
                nxt = emit_a(j)
                v_transposes(*pend)
                pend = nxt
            nxt = emit_a(4)
            v_transposes(*pend)  # tpose(3): batch 0 V complete
            pend = nxt
            for j, qc in ((5, 0), (6, 1), (7, 2)):
                emit_b(0, 0, qc)
                nxt = emit_a(j)
                v_transposes(*pend)
                pend = nxt
            emit_b(0, 0, 3)
            flush_norms()
            v_transposes(*pend)  # tpose(7): batch 1 V complete
            for qc in range(4):
                emit_b(0, 1, qc)
            flush_norms()
            emit_a2a(0)
            for b in range(B):
                for qc in range(4):
                    emit_b(1, b, qc)
                    if b == 1 and qc == 2:
                        # drain all but the last chunk's normalization so
                        # only qc3's chain sits on the A2A#1 critical path
                        flush_norms()
                flush_norms()
                if b == 0:
                    # pre-send the batch-0 half of the h1 A2A payload
                    nc.sync.dma_start(
                        a2a_in[1][0:4].transpose([1, 0, 2]),
                        att_out[:, 1, 0:T].rearrange("p (s q) -> p s q", s=4),
                    )
            # h0's projection half runs while the second A2A is in flight;
            # emitted before the a2a so a coarse dep on att_all can't stall
            # it (the PE only needs partitions 0-63, written by A2A#0)
            emit_a2a(1, half_only=True)
            for oc in range(2):
                for s in range(4):
                    ps_o = proj_half(0, oc, s)
                    nc.vector.tensor_copy(
                        ostage[:, s, oc * 512:(oc + 1) * 512], ps_o[:])
            # keep the PE's activity monitor warm through the A2A wait so
            # the final projection runs at full clock; the result is written
            # to a scratch DRAM tile so DCE keeps the matmuls
            warm_scratch = dpool.tile([128, 512], F32, tag="warm",
                                      name="warm_scratch")
            ps_w = ppool.tile([128, 512], F32, tag="pS", bufs=3, name="ps_w")
            for i in range(24):
                nc.tensor.matmul(ps_w[:], rot_sb[:], cos_sb[:, 0:512],
                                 start=(i == 0), stop=(i == 23))
            w_sb2 = wpool.tile([128, 512], F32, tag="osb", bufs=1,
                               name="w_sb2")
            nc.vector.tensor_copy(w_sb2[:], ps_w[:])
            nc.sync.dma_start(warm_scratch[:], w_sb2[:])

            if debug:
                nc.sync.dma_start(dbg_qrope[:], qrope[:].bitcast(F32))
                nc.sync.dma_start(
                    dbg_krope[:],
                    krope[:].rearrange("p a b -> p (a b)").bitcast(F32))
                nc.sync.dma_start(
                    dbg_vall[:],
                    v_all[:].rearrange("p a b c d -> p (a b c d)")
                    .bitcast(F32))
                nc.gpsimd.dma_start(
                    dbg_attout[:],
                    att_out[:].rearrange("p a b -> p (a b)"))
                for h in range(HPC):
                    hs_ = slice(h * 64, (h + 1) * 64)
                    nc.gpsimd.dma_start(
                        dbg_attall[hs_, :],
                        att_alls[h][hs_, :, :].rearrange("p a b -> p (a b)"))

            # ---- Phase C tail: h1 half + combine + store ----
            for oc in range(2):
                for s in range(4):
                    ps_o = proj_half(1, oc, s)
                    o_sb = wpool.tile([128, 512], F32, tag="osb", bufs=1)
                    nc.vector.tensor_add(
                        o_sb[:], ps_o[:],
                        ostage[:, s, oc * 512:(oc + 1) * 512])
                    nc.sync.dma_start(
                        out[s * 128:(s + 1) * 128, oc * 512:(oc + 1) * 512],
                        o_sb[:],
                    )
    nc.compile()
    return nc


def _prep_in_maps(x, wq, wk, wv, wo, cos, sin, mask):
    import ml_dtypes
    # xt[j, p, c, t] = x[j*512 + t, c*128 + p]
    xt = np.ascontiguousarray(
        x.reshape(NTC, 512, NF, 128).transpose(0, 3, 2, 1)).astype(np.float32)
    # wo_t[p, c, o] = wo.T[c*128 + p, o] = wo[o, c*128 + p]
    wo_t = np.ascontiguousarray(
        wo.T.reshape(NF, 128, D).transpose(1, 0, 2)).astype(ml_dtypes.bfloat16)
    cos2 = np.ascontiguousarray(np.tile(cos.T, (HPC, 1))).astype(np.float32)
    sin2 = np.ascontiguousarray(np.tile(sin.T, (HPC, 1))).astype(np.float32)
    rot2t = np.ascontiguousarray(_rot_matrix().T)
    ident = np.eye(128, dtype=np.float32)
    trimask = np.ascontiguousarray(mask[0, 0, :128, :128].T).astype(np.float32)
    scale = HD ** -0.5
    in_maps = []
    for c in range(N_CORES):
        rows = slice(c * 128, (c + 1) * 128)
        in_maps.append({
            "xt": xt,
            "wq_t": np.ascontiguousarray(
                (wq[rows, :] * scale).T.reshape(NF, 128, 128)
                .transpose(1, 0, 2)).astype(np.float32),
            "wk_t": np.ascontiguousarray(
                wk[rows, :].T.reshape(NF, 128, 128)
                .transpose(1, 0, 2)).astype(np.float32),
            "wv_t": np.ascontiguousarray(
                wv[rows, :].T.reshape(NF, 128, 128)
                .transpose(1, 0, 2)).astype(np.float32),
            "wo_t": wo_t,
            "cos2": cos2,
            "sin2": sin2,
            "rot2t": rot2t,
            "ident": ident,
            "trimask": trimask,
        })
    return in_maps


def kernel(x, wq, wk, wv, wo, cos, sin, mask, _trace=False, _debug=False):
    x, wq, wk, wv, wo = (np.asarray(a, dtype=np.float32)
                         for a in (x, wq, wk, wv, wo))
    cos, sin = np.asarray(cos, dtype=np.float32), np.asarray(sin, dtype=np.float32)
    mask = np.asarray(mask)
    key = ("nc", _debug)
    if key not in _CACHE:
        _CACHE[key] = build(debug=_debug)
    nc = _CACHE[key]
    in_maps = _prep_in_maps(x, wq, wk, wv, wo, cos, sin, mask)
    res = bass_utils.run_bass_kernel_spmd(
        nc, in_maps, core_ids=list(range(N_CORES)), trace=_trace)
    _CACHE["last_result"] = res
    full = np.concatenate([res.results[c]["out"] for c in range(N_CORES)], axis=0)
    return full.reshape(B, T, D).astype(np.float32)


# revision 36
# speedup vs baseline: 1.0394x; 1.0394x over previous
"""Multi-head causal attention with RoPE on 8 TRN2 NeuronCores.

Problem: B=2, T=2048, D=1024, H=16 heads, head_dim=64.
  out = softmax(mask(rope(x@Wq.T) @ rope(x@Wk.T).T / 8)) @ (x@Wv.T) @ Wo.T

Sharding: tensor-parallel over heads. Core c owns heads {2c, 2c+1}:
  - computes Q/K/V projections for its 2 heads over all 4096 tokens
    (x pre-transposed/pre-tiled on host; the 1/sqrt(hd) scale is folded
    into Wq),
  - RoPE via a block-diagonal rotation matmul + cos/sin elementwise,
  - causal flash-style attention in transposed layout (scores^T [k, q]
    tiles, exp on ScalarE, lower-triangle tiles only, row-sums via an
    appended ones-column on V),
  - an AllToAll (2 x 1MB bf16, split by local head) redistributes
    attention outputs from head-sharded to row-sharded,
  - row-parallel output projection, contracted in two K=64 halves so
    the first half overlaps the second AllToAll: core c computes output
    rows [512c, 512(c+1)) of the flattened [4096, 1024] output.

Matmuls run as float32r (fp32 storage, TF32-like multiply, 1 cyc/row at
N>=256); PSUM accumulates fp32. Two hardware quirks shape the code: the
PE only pipelines back-to-back matmuls whose contraction dim K matches
(K-reconfig flushes it), so scores matmuls use a per-head zero-padded
rope(K) to stay K=128 and V-transposes are batched; and the projection
phase (PE-heavy, idle ScalarE) is interleaved with the attention phase
(ScalarE-exp-bound) so both engines stay busy.
"""
import sys

sys.path.insert(0, "/opt/trn_rl_repo")

import numpy as np

from concourse import bacc, mybir, tile
from concourse import bass_utils

N_CORES = 8
B, T, D, H = 2, 2048, 1024, 16
HD = D // H              # 64
HPC = H // N_CORES       # 2 heads per core
BT = B * T               # 4096
NF = D // 128            # 8 feature chunks
NTC = BT // 512          # 8 t-chunks of 512
QCHUNK = 512
ROWS_PER_CORE = BT // N_CORES  # 512 output rows per core

F32 = mybir.dt.float32
F32R = mybir.dt.float32r
BF16 = mybir.dt.bfloat16

_CACHE = {}


def _rot_matrix():
    """R2 = blockdiag(R, R), R@u = rotate_half(u) per 64-dim head."""
    half = HD // 2
    R = np.zeros((HD, HD), dtype=np.float32)
    for i in range(half):
        R[i, i + half] = -1.0
        R[i + half, i] = 1.0
    R2 = np.zeros((2 * HD, 2 * HD), dtype=np.float32)
    R2[:HD, :HD] = R
    R2[HD:, HD:] = R
    return R2


def build(debug=False):
    nc = bacc.Bacc("TRN2", target_bir_lowering=False, debug=False,
                   num_devices=N_CORES)

    # ---- DRAM parameters (per-core shards, host-prepped layouts) ----
    xt = nc.declare_dram_parameter("xt", [NTC, 128, NF, 512], F32, isOutput=False)
    wq_t = nc.declare_dram_parameter("wq_t", [128, NF, 128], F32, isOutput=False)
    wk_t = nc.declare_dram_parameter("wk_t", [128, NF, 128], F32, isOutput=False)
    wv_t = nc.declare_dram_parameter("wv_t", [128, NF, 128], F32, isOutput=False)
    wo_t = nc.declare_dram_parameter("wo_t", [128, NF, D], BF16, isOutput=False)
    cos2 = nc.declare_dram_parameter("cos2", [128, T], F32, isOutput=False)
    sin2 = nc.declare_dram_parameter("sin2", [128, T], F32, isOutput=False)
    rot2t = nc.declare_dram_parameter("rot2t", [128, 128], F32, isOutput=False)
    ident = nc.declare_dram_parameter("ident", [128, 128], F32, isOutput=False)
    trimask = nc.declare_dram_parameter("trimask", [128, 128], F32, isOutput=False)
    out = nc.declare_dram_parameter("out", [ROWS_PER_CORE, D], F32, isOutput=True)
    if debug:
        dbg_qrope = nc.declare_dram_parameter("dbg_qrope", [128, BT], F32, isOutput=True)
        dbg_krope = nc.declare_dram_parameter("dbg_krope", [128, HPC * BT], F32, isOutput=True)
        dbg_vall = nc.declare_dram_parameter("dbg_vall", [128, B * HPC * 16 * 65], F32, isOutput=True)
        dbg_attout = nc.declare_dram_parameter("dbg_attout", [64, HPC * BT], F32, isOutput=True)
        dbg_attall = nc.declare_dram_parameter("dbg_attall", [128, N_CORES * QCHUNK], F32, isOutput=True)

    with tile.TileContext(nc) as tc, nc.allow_low_precision(reason="f32r compute"):
        with (
            tc.tile_pool(name="consts", bufs=1) as cpool,
            tc.tile_pool(name="work", bufs=1) as wpool,
            tc.tile_pool(name="psum", bufs=1, space="PSUM") as ppool,
            tc.tile_pool(name="dram", bufs=1, space="DRAM") as dpool,
        ):
            # ---- persistent tensors ----
            rot_sb = cpool.tile([128, 128], F32R, tag="rot")
            id_sb = cpool.tile([128, 128], F32R, tag="ident")
            tri_sb = cpool.tile([128, 128], F32R, tag="tri")
            cos_sb = cpool.tile([128, T], F32R, tag="cos")
            sin_sb = cpool.tile([128, T], F32R, tag="sin")
            wo_sb = cpool.tile([128, NF, D], BF16, tag="wo")
            qrope = cpool.tile([128, BT], F32R, tag="qrope")
            # K rope, zero-padded per head so scores matmuls keep K=128
            krope = cpool.tile([128, HPC, BT], F32R, tag="krope")
            # V per (b, h): [128 t-part, 16 t-tiles, 65] (col 64 = ones)
            v_all = cpool.tile([128, B, HPC, T // 128, 65], F32R, tag="v_all")
            att_out = cpool.tile([64, HPC, BT], BF16, tag="att_out")
            att_alls = [cpool.tile([128, N_CORES, QCHUNK], BF16,
                                   tag=f"att_all{h}", name=f"att_all{h}")
                        for h in range(HPC)]
            ostage = cpool.tile([128, 4, D], BF16, tag="ostage")

            wq_sb = cpool.tile([128, NF, 128], F32R, tag="wq")
            wk_sb = cpool.tile([128, NF, 128], F32R, tag="wk")
            wv_sb = cpool.tile([128, NF, 128], F32R, tag="wv")

            a2a_in = [dpool.tile([N_CORES, 64, 512], BF16, tag=f"a2a_in{h}",
                                 name=f"a2a_in{h}")
                      for h in range(HPC)]
            a2a_out = [dpool.tile([N_CORES, 64, 512], BF16, tag=f"a2a_out{h}",
                                  name=f"a2a_out{h}")
                       for h in range(HPC)]

            # ---- DMA loads; order matters: the first projection matmul
            # needs only wq f-chunk 0 + xt chunk-0 f-chunk 0 (~320KB), so
            # those are split into their own transfers to start the PE early
            nc.sync.dma_start(wq_sb[:, 0:1, :], wq_t[:, 0:1, :].bitcast(F32R))

            def load_xt_half(j, half, nsplit=1):
                """One 512-token, 4-feature-chunk half of x^T (contiguous)."""
                xh = wpool.tile([128, NF // 2, 512], F32R, tag="xt", bufs=3,
                                name="xh")
                c0 = half * 4
                step = 4 // nsplit
                for cc in range(0, 4, step):
                    nc.sync.dma_start(
                        xh[:, cc:cc + step, :],
                        xt[j, :, c0 + cc:c0 + cc + step, :].bitcast(F32R))
                return xh

            xt_pre = [load_xt_half(0, 0, nsplit=4), load_xt_half(0, 1)]
            nc.sync.dma_start(wq_sb[:, 1:8, :], wq_t[:, 1:8, :].bitcast(F32R))
            nc.sync.dma_start(wk_sb[:], wk_t[:].bitcast(F32R))
            nc.sync.dma_start(wv_sb[:], wv_t[:].bitcast(F32R))
            nc.scalar.dma_start(rot_sb[:], rot2t[:].bitcast(F32R))
            nc.scalar.dma_start(cos_sb[:], cos2[:].bitcast(F32R))
            nc.scalar.dma_start(sin_sb[:], sin2[:].bitcast(F32R))
            nc.scalar.dma_start(id_sb[:], ident[:].bitcast(F32R))
            nc.scalar.dma_start(tri_sb[:], trimask[:].bitcast(F32R))
            nc.gpsimd.dma_start(wo_sb[:], wo_t[:])

            # zero the pad halves of krope; ones col of v_all
            nc.vector.memset(krope[64:128, 0, :].bitcast(F32), 0.0)
            nc.vector.memset(krope[0:64, 1, :].bitcast(F32), 0.0)
            nc.vector.memset(v_all[:, :, :, :, 64].bitcast(F32), 1.0)

            # ---- phase A pieces ----
            def emit_a(j, xh01=None):
                """Projection chunk j; returns state for rope/vt laggards."""
                xh = xh01 or [load_xt_half(j, 0), load_xt_half(j, 1)]
                ps_q = ppool.tile([128, 512], F32, tag="pP", bufs=3)
                ps_k = ppool.tile([128, 512], F32, tag="pP", bufs=3,
                                  name="ps_k")
                ps_v = ppool.tile([128, 512], F32, tag="pP", bufs=3,
                                  name="ps_v")
                for f in range(NF):
                    st, sp = (f == 0), (f == NF - 1)
                    src = xh[f // 4][:, f % 4, :]
                    nc.tensor.matmul(ps_q[:], wq_sb[:, f, :], src,
                                     start=st, stop=sp)
                    nc.tensor.matmul(ps_k[:], wk_sb[:, f, :], src,
                                     start=st, stop=sp)
                    nc.tensor.matmul(ps_v[:], wv_sb[:, f, :], src,
                                     start=st, stop=sp)
                qT = wpool.tile([128, 512], F32R, tag="qT", bufs=1)
                kT = wpool.tile([128, 512], F32R, tag="kT", bufs=1)
                vT = wpool.tile([128, 512], F32R, tag="vT", bufs=2)
                nc.vector.tensor_copy(qT[:], ps_q[:])
                nc.vector.tensor_copy(kT[:], ps_k[:])
                nc.vector.tensor_copy(vT[:], ps_v[:])
                # rotation matmuls (same K=128/N=512 shape as projections)
                ps_rq = ppool.tile([128, 512], F32, tag="pB", bufs=2,
                                   name="ps_rq")
                nc.tensor.matmul(ps_rq[:], rot_sb[:], qT[:],
                                 start=True, stop=True)
                ps_rk = ppool.tile([128, 512], F32, tag="pB", bufs=2,
                                   name="ps_rk")
                nc.tensor.matmul(ps_rk[:], rot_sb[:], kT[:],
                                 start=True, stop=True)
                # rope combines on DVE
                tl = (j % 4) * 512
                J = slice(j * 512, (j + 1) * 512)
                TL = slice(tl, tl + 512)
                tmp = wpool.tile([128, 512], F32R, tag="ropetmp", bufs=2,
                                 name="tmp")
                nc.vector.tensor_mul(tmp[:], ps_rq[:], sin_sb[:, TL])
                nc.vector.tensor_mul(qrope[:, J], qT[:], cos_sb[:, TL])
                nc.vector.tensor_add(qrope[:, J], qrope[:, J], tmp[:])
                tmpk = wpool.tile([128, 512], F32R, tag="ropetmp", bufs=2,
                                  name="tmpk")
                nc.vector.tensor_mul(tmpk[:], ps_rk[:], sin_sb[:, TL])
                for h in range(HPC):
                    hs = slice(h * 64, (h + 1) * 64)
                    nc.vector.tensor_mul(krope[hs, h, J], kT[hs, :],
                                         cos_sb[hs, TL])
                    nc.vector.tensor_add(krope[hs, h, J], krope[hs, h, J],
                                         tmpk[hs, :])
                return (j, vT)

            def v_transposes(j, vT):
                b = j // 4
                for h in range(HPC):
                    hs = slice(h * 64, (h + 1) * 64)
                    for tt in range(4):
                        ps_t = ppool.tile([128, 64], F32R, tag="pB",
                                          bufs=2, name="ps_t")
                        nc.tensor.transpose(
                            ps_t[:, :],
                            vT[hs, tt * 128:(tt + 1) * 128],
                            id_sb[hs, hs],
                        )
                        nc.scalar.copy(
                            v_all[:, b, h, (j % 4) * 4 + tt, 0:64], ps_t[:])

            # ---- phase B pieces ----
            def scores_mm(h, base, q0, kt):
                k0 = kt * 128
                ps_s = ppool.tile([128, 512], F32, tag="pS", bufs=3,
                                  name="ps_s")
                nc.tensor.matmul(
                    ps_s[:],
                    krope[:, h, base + k0:base + k0 + 128],
                    qrope[:, base + q0:base + q0 + 512],
                    start=True, stop=True,
                )
                return ps_s

            def exp_mask(ps_s, n_full, kt):
                ae = wpool.tile([128, 512], F32R, tag="attexp", bufs=3,
                                name="ae")
                if kt < n_full:
                    nc.scalar.activation(
                        ae[:], ps_s[:], mybir.ActivationFunctionType.Exp)
                else:
                    v = kt - n_full
                    nc.scalar.activation(
                        ae[:, v * 128:512], ps_s[:, v * 128:512],
                        mybir.ActivationFunctionType.Exp)
                    nc.vector.tensor_mul(
                        ae[:, v * 128:(v + 1) * 128],
                        ae[:, v * 128:(v + 1) * 128],
                        tri_sb[:],
                    )
                    if v > 0:
                        nc.vector.memset(ae[:, 0:v * 128].bitcast(F32), 0.0)
                return ae

            def emit_b(h, b, qc):
                base = b * T
                q0 = qc * QCHUNK
                n_full = q0 // 128
                n_kt = n_full + 4
                attv = ppool.tile([65, 512], F32, tag="pB", bufs=2)
                PIPE = 3
                pend_s = [scores_mm(h, base, q0, kt)
                          for kt in range(min(PIPE, n_kt))]
                for kt in range(n_kt):
                    ae = exp_mask(pend_s[kt], n_full, kt)
                    if kt + PIPE < n_kt:
                        pend_s.append(scores_mm(h, base, q0, kt + PIPE))
                    nc.tensor.matmul(
                        attv[:], v_all[:, b, h, kt, :], ae[:],
                        start=(kt == 0), stop=(kt == n_kt - 1),
                    )
                # stage the unnormalized output + reciprocal of row-sums,
                # releasing the PSUM slot via DVE only; the DMA/broadcast/
                # multiply normalization chain is deferred (flushed in
                # batches) so its latency never blocks the DVE queue that
                # feeds attention.
                J = slice(base + q0, base + q0 + 512)
                nc.vector.tensor_copy(att_out[:, h, J], attv[0:64, :])
                rcp = wpool.tile([65, 512], F32, tag="rcp", bufs=2)
                # copy the sum row out first (fast) so the PSUM slot frees
                # without waiting for the slow reciprocal
                nc.vector.tensor_copy(rcp[64:65, :], attv[64:65, :])
                nc.vector.reciprocal(rcp[64:65, :], rcp[64:65, :])

                def norm(h=h, J=J, rcp=rcp):
                    rcp0 = wpool.tile([1, 512], F32, tag="rcp0", bufs=2)
                    nc.sync.dma_start(rcp0[:], rcp[64:65, :])
                    brcp = wpool.tile([64, 512], F32, tag="brcp", bufs=3)
                    nc.gpsimd.partition_broadcast(brcp[:], rcp0[:])
                    nc.vector.tensor_mul(
                        att_out[:, h, J], att_out[:, h, J], brcp[:])
                deferred_norms.append(norm)

            def flush_norms():
                for fn in deferred_norms:
                    fn()
                deferred_norms.clear()

            def emit_a2a(h, half_only=False):
                hs = slice(h * 64, (h + 1) * 64)
                if half_only:
                    nc.sync.dma_start(
                        a2a_in[h][4:8].transpose([1, 0, 2]),
                        att_out[:, h, T:].rearrange("p (s q) -> p s q", s=4),
                    )
                else:
                    nc.sync.dma_start(
                        a2a_in[h][:].transpose([1, 0, 2]),
                        att_out[:, h, :].rearrange("p (s q) -> p s q",
                                                   s=N_CORES),
                    )
                nc.gpsimd.collective_compute(
                    "AllToAll", mybir.AluOpType.bypass,
                    replica_groups=[list(range(N_CORES))],
                    ins=[a2a_in[h].opt()],
                    outs=[a2a_out[h].opt()],
                )
                # on gpsimd so this long-waiting DMA never heads the Sync
                # queue in front of small latency-critical transfers; each
                # head has its own tile so the h0 projection can't pick up
                # a false dependency on the h1 A2A write
                nc.gpsimd.dma_start(
                    att_alls[h][hs, :, :],
                    a2a_out[h][:].transpose([1, 0, 2]),
                )

            def proj_half(hl, oc, s):
                lo = slice(hl * 64, (hl + 1) * 64)
                ps_o = ppool.tile([128, 512], F32, tag="pP", bufs=3,
                                  name="ps_o")
                for c in range(N_CORES):
                    nc.tensor.matmul(
                        ps_o[:],
                        att_alls[hl][lo, c, s * 128:(s + 1) * 128],
                        wo_sb[lo, c, oc * 512:(oc + 1) * 512],
                        start=(c == 0), stop=(c == N_CORES - 1),
                    )
                return ps_o

            # ---- interleaved schedule ----
            # A chunks 0-3 produce batch 0; B(h0,b0) then interleaves with
            # A chunks 4-7 (batch 1), keeping ScalarE and PE both busy.
            deferred_norms = []
            pend = emit_a(0, xt_pre)
            for j in range(1, 4):
                nxt = emit_a(j)
                v_transposes(*pend)
                pend = nxt
            nxt = emit_a(4)
            v_transposes(*pend)  # tpose(3): batch 0 V complete
            pend = nxt
            for j, qc in ((5, 0), (6, 1), (7, 2)):
                emit_b(0, 0, qc)
                nxt = emit_a(j)
                v_transposes(*pend)
                pend = nxt
            emit_b(0, 0, 3)
            flush_norms()
            v_transposes(*pend)  # tpose(7): batch 1 V complete
            for qc in range(4):
                emit_b(0, 1, qc)
            flush_norms()
            emit_a2a(0)
            for b in range(B):
                for qc in range(4):
                    emit_b(1, b, qc)
                    if b == 1 and qc == 2:
                        # drain all but the last chunk's normalization so
                        # only qc3's chain sits on the A2A#1 critical path
                        flush_norms()
                flush_norms()
                if b == 0:
                    # pre-send the batch-0 half of the h1 A2A payload
                    nc.sync.dma_start(
                        a2a_in[1][0:4].transpose([1, 0, 2]),
                        att_out[:, 1, 0:T].rearrange("p (s q) -> p s q", s=4),
                    )
            # h0's projection half runs while the second A2A is in flight;
            # emitted before the a2a so a coarse dep on att_all can't stall
            # it (the PE only needs partitions 0-63, written by A2A#0)
            emit_a2a(1, half_only=True)
            for oc in range(2):
                for s in range(4):
                    ps_o = proj_half(0, oc, s)
                    nc.vector.tensor_copy(
                        ostage[:, s, oc * 512:(oc + 1) * 512], ps_o[:])
            # keep the PE's activity monitor warm through the A2A wait so
            # the final projection runs at full clock; the result is written
            # to a scratch DRAM tile so DCE keeps the matmuls
            warm_scratch = dpool.tile([128, 512], F32, tag="warm",
                                      name="warm_scratch")
            ps_w = ppool.tile([128, 512], F32, tag="pS", bufs=3, name="ps_w")
            for i in range(24):
                nc.tensor.matmul(ps_w[:], rot_sb[:], cos_sb[:, 0:512],
                                 start=(i == 0), stop=(i == 23))
            w_sb2 = wpool.tile([128, 512], F32, tag="osb", bufs=1,
                               name="w_sb2")
            nc.vector.tensor_copy(w_sb2[:], ps_w[:])
            nc.sync.dma_start(warm_scratch[:], w_sb2[:])

            if debug:
                nc.sync.dma_start(dbg_qrope[:], qrope[:].bitcast(F32))
                nc.sync.dma_start(
                    dbg_krope[:],
                    krope[:].rearrange("p a b -> p (a b)").bitcast(F32))
                nc.sync.dma_start(
                    dbg_vall[:],
                    v_all[:].rearrange("p a b c d -> p (a b c d)")
                    .bitcast(F32))
                nc.gpsimd.dma_start(
                    dbg_attout[:],
                    att_out[:].rearrange("p a b -> p (a b)"))
                for h in range(HPC):
                    hs_ = slice(h * 64, (h + 1) * 64)
                    nc.gpsimd.dma_start(
                        dbg_attall[hs_, :],
                        att_alls[h][hs_, :, :].rearrange("p a b -> p (a b)"))

            # ---- Phase C tail: h1 half + combine + store ----
            for oc in range(2):
                for s in range(4):
                    ps_o = proj_half(1, oc, s)
                    o_sb = wpool.tile([128, 512], F32, tag="osb", bufs=1)
                    nc.vector.tensor_add(
                        o_sb[:], ps_o[:],
                        ostage[:, s, oc * 512:(oc + 1) * 512])
                    nc.sync.dma_start(
                        out[s * 128:(s + 1) * 128, oc * 512:(oc + 1) * 512],
                        o_sb[:],
                    )
    nc.compile()
    return nc


def _prep_in_maps(x, wq, wk, wv, wo, cos, sin, mask):
    import ml_dtypes
    # xt[j, p, c, t] = x[j*512 + t, c*128 + p]
    xt = np.ascontiguousarray(
        x.reshape(NTC, 512, NF, 128).transpose(0, 3, 2, 1)).astype(np.float32)
    # wo_t[p, c, o] = wo.T[c*128 + p, o] = wo[o, c*128 + p]
    wo_t = np.ascontiguousarray(
        wo.T.reshape(NF, 128, D).transpose(1, 0, 2)).astype(ml_dtypes.bfloat16)
    cos2 = np.ascontiguousarray(np.tile(cos.T, (HPC, 1))).astype(np.float32)
    sin2 = np.ascontiguousarray(np.tile(sin.T, (HPC, 1))).astype(np.float32)
    rot2t = np.ascontiguousarray(_rot_matrix().T)
    ident = np.eye(128, dtype=np.float32)
    trimask = np.ascontiguousarray(mask[0, 0, :128, :128].T).astype(np.float32)
    scale = HD ** -0.5
    in_maps = []
    for c in range(N_CORES):
        rows = slice(c * 128, (c + 1) * 128)
        in_maps.append({
            "xt": xt,
            "wq_t": np.ascontiguousarray(
                (wq[rows, :] * scale).T.reshape(NF, 128, 128)
                .transpose(1, 0, 2)).astype(np.float32),
            "wk_t": np.ascontiguousarray(
                wk[rows, :].T.reshape(NF, 128, 128)
                .transpose(1, 0, 2)).astype(np.float32),
            "wv_t": np.ascontiguousarray(
                wv[rows, :].T.reshape(NF, 128, 128)
                .transpose(1, 0, 2)).astype(np.float32),
            "wo_t": wo_t,
            "cos2": cos2,
            "sin2": sin2,
            "rot2t": rot2t,
            "ident": ident,
            "trimask": trimask,
        })
    return in_maps


def kernel(x, wq, wk, wv, wo, cos, sin, mask, _trace=False, _debug=False):
    x, wq, wk, wv, wo = (np.asarray(a, dtype=np.float32)
                         for a in (x, wq, wk, wv, wo))
    cos, sin = np.asarray(cos, dtype=np.float32), np.asarray(sin, dtype=np.float32)
    mask = np.asarray(mask)
    key = ("nc", _debug)
    if key not in _CACHE:
        _CACHE[key] = build(debug=_debug)
    nc = _CACHE[key]
    in_maps = _prep_in_maps(x, wq, wk, wv, wo, cos, sin, mask)
    res = bass_utils.run_bass_kernel_spmd(
        nc, in_maps, core_ids=list(range(N_CORES)), trace=_trace)
    _CACHE["last_result"] = res
    full = np.concatenate([res.results[c]["out"] for c in range(N_CORES)], axis=0)
    return full.reshape(B, T, D).astype(np.float32)
